# revision 2
# baseline (speedup 1.0000x reference)
"""Trainium2 Bass kernel v2: 2-layer GCN + mean-pool + GRU + layernorm + linear.

Key design vs v1:
- Interleaved banking: table row for node v is pid = core*NPC + pos, window
  b = rows with pid%4==b (elem_step = 4 rows). A node's bank as a *source* is
  pos%4, which is chosen by a host-side greedy coloring that balances each
  destination's in-edges across the 4 banks (pad ~1.3x vs 2.17x for v1's
  fixed core-block banking).
- dinv folded into the table (x' = dinv*x, h1' = dinv*h1): no per-edge norm
  multiply, no nrm stream; padding slots gather reserved always-zero rows.
- Fold matrix F per tile replaces the transpose identity: maps gather rows ->
  node columns (handles ghost cols and split nodes), accumulated over the 4
  bank partial-reduces in PSUM. Same PE cost as v1's transpose.
- bf16 gather tables (HBM random reads are byte-bound), f32 compute
  (reduce/fold/matmuls); measured rel err ~2e-3 vs 2e-2 tolerance.
- Gather calls grouped over runs of equal-K tiles, split at 2048 idx
  (single_packet=False), rotated across 4 SWDGE queues.
"""

import os as _os
import numpy as np
from collections import deque

N_NODES = 100000
N_EDGES = 1600000
N_GRAPHS = 2048
D = 128
P = 128
EPS = 1e-5
NC = 8
GPC = N_GRAPHS // NC
NB = 4
MAXIDX = int(_os.environ.get("MAXIDX", "2048"))
KCAP = 24
MAX_GT = 6                     # tiles per gather group
MAX_GSLOT = 24                 # slots (per bank) per gather group
NSWEEP = int(_os.environ.get("NSWEEP", "2"))

_CACHE = {}
_last_in_maps = None


def _prep(x, src, dst, batch):
    deg = np.bincount(dst, minlength=N_NODES).astype(np.float64) + 1.0
    dinv = (1.0 / np.sqrt(deg)).astype(np.float32)

    node_start = np.searchsorted(batch, np.arange(0, N_GRAPHS + 1, GPC))
    core_of = np.searchsorted(node_start, np.arange(N_NODES), side="right") - 1

    a_src = np.concatenate([src, np.arange(N_NODES)])
    a_dst = np.concatenate([dst, np.arange(N_NODES)])

    # ---- source coloring: balance each dst's in-edges across 4 banks ----
    # Batched greedy: each round, every source scores each color by the sum of
    # its out-neighbors' current color counts; a random subset of improving
    # sources flips. Minimizes sum cnt^2 (proxy for per-dst max-bank count).
    rng = np.random.default_rng(0)
    color = rng.integers(0, NB, N_NODES).astype(np.int64)
    cnt = np.zeros((N_NODES, NB), np.float64)
    np.add.at(cnt, (a_dst, color[a_src]), 1)
    # sequential greedy sweeps (pure python; batch rounds herd/oscillate)
    order = np.argsort(a_src, kind="stable")
    odst_l = a_dst[order].tolist()
    optr_l = np.searchsorted(a_src[order], np.arange(N_NODES + 1)).tolist()
    cnt_l = cnt.astype(np.int64).ravel().tolist()
    color_l = color.tolist()
    for _ in range(NSWEEP):
        for v in range(N_NODES):
            lo, hi = optr_l[v], optr_l[v + 1]
            if lo == hi:
                continue
            c_old = color_l[v]
            c0 = c1 = c2 = c3 = 0
            m0 = m1 = m2 = m3 = 0
            for j in range(lo, hi):
                u4 = odst_l[j] * 4
                a = cnt_l[u4]
                b = cnt_l[u4 + 1]
                cc = cnt_l[u4 + 2]
                d = cnt_l[u4 + 3]
                if c_old == 0:
                    a -= 1
                elif c_old == 1:
                    b -= 1
                elif c_old == 2:
                    cc -= 1
                else:
                    d -= 1
                c0 += a
                c1 += b
                c2 += cc
                c3 += d
                if a > m0:
                    m0 = a
                if b > m1:
                    m1 = b
                if cc > m2:
                    m2 = cc
                if d > m3:
                    m3 = d
            s0, s1, s2, s3 = c0 + m0, c1 + m1, c2 + m2, c3 + m3
            c_new = 0
            best = s0
            if s1 < best:
                best = s1
                c_new = 1
            if s2 < best:
                best = s2
                c_new = 2
            if s3 < best:
                best = s3
                c_new = 3
            if c_new != c_old:
                for j in range(lo, hi):
                    u4 = odst_l[j] * 4
                    cnt_l[u4 + c_old] -= 1
                    cnt_l[u4 + c_new] += 1
                color_l[v] = c_new
    color = np.array(color_l, np.int64)
    cnt = np.array(cnt_l, np.float64).reshape(N_NODES, NB)
    if _os.environ.get("PREPDBG"):
        print("after sweeps: mean max-bank", cnt.max(axis=1).mean(),
              "ideal", np.ceil((np.bincount(a_dst, minlength=N_NODES)) / NB).mean())

    # balance colors within each sorted 128-window (tile caps are 32/color):
    # flip excess nodes to underfull colors so packing never defers. Counts
    # are updated incrementally so flips don't herd onto shared destinations.
    cnt_i = cnt.astype(np.int64)
    odst_np = np.asarray(odst_l, np.int64)
    optr_np = np.asarray(optr_l, np.int64)
    for c in range(NC):
        lo, hi = node_start[c], node_start[c + 1]
        nodes = np.arange(lo, hi)
        n = nodes.size
        tgt = n // NB
        for _ in range(2 * NB):
            counts = np.bincount(color[nodes], minlength=NB)
            ocol = int(np.argmax(counts))
            ucol = int(np.argmin(counts))
            mv = int(min(counts[ocol] - tgt - 1, tgt - counts[ucol]))
            if mv <= 0:
                break
            cand = nodes[color[nodes] == ocol]
            deltas = np.array([
                (cnt_i[odst_np[optr_np[v]:optr_np[v + 1]], ucol]
                 - cnt_i[odst_np[optr_np[v]:optr_np[v + 1]], ocol]).sum()
                for v in cand])
            pick = cand[np.argsort(deltas)[:mv]]
            for v in pick:
                us = odst_np[optr_np[v]:optr_np[v + 1]]
                cnt_i[us, ocol] -= 1
                cnt_i[us, ucol] += 1
            color[pick] = ucol
    cnt = cnt_i
    # final sort by post-rebalance kmax (packing handles small color-cap
    # deferrals via its pending queue)
    per_core_nodes = []
    for c in range(NC):
        lo, hi = node_start[c], node_start[c + 1]
        nodes = np.arange(lo, hi)
        o = np.argsort(-cnt[nodes].max(axis=1), kind="stable")
        per_core_nodes.append(nodes[o])
    if _os.environ.get("PREPDBG"):
        print("after flips: mean max-bank", cnt.max(axis=1).mean())

    tiles_max = 127
    usedK = [0] * tiles_max
    assigns = []
    kmax_all = cnt.max(axis=1)
    color_l = color.tolist()
    CAP = P // NB
    for c in range(NC):
        nodes = per_core_nodes[c]
        kmax_l = kmax_all[nodes].tolist()
        nlist = nodes.tolist()
        n = len(nlist)
        asg = np.zeros((n, 4), np.int64)  # tile, col, row0, nrows
        queue = deque(range(n))            # indices into nlist
        pending = deque()
        t, rows, ncols = 0, 0, 0
        Kt = 0
        colcnt = [0, 0, 0, 0]
        while queue or pending:
            i = None
            for _ in range(len(pending)):
                j = pending.popleft()
                if colcnt[color_l[nlist[j]]] < CAP:
                    i = j
                    break
                pending.append(j)
            if i is None and queue:
                j = queue.popleft()
                if colcnt[color_l[nlist[j]]] < CAP:
                    i = j
                else:
                    pending.append(j)
                    continue
            if i is None:
                t += 1
                rows = ncols = 0
                Kt = 0
                colcnt = [0, 0, 0, 0]
                assert t < tiles_max, "out of tiles (color)"
                continue
            kmax = kmax_l[i]
            if Kt == 0:
                # tile K from the max over the next ~window of candidates
                peek = kmax
                for j in list(pending)[:P]:
                    if kmax_l[j] > peek:
                        peek = kmax_l[j]
                lim = min(len(queue), P)
                for jj in range(lim):
                    j = queue[jj]
                    if kmax_l[j] > peek:
                        peek = kmax_l[j]
                Kt = max(1, min(peek, KCAP))
            need = max(1, -(-kmax // Kt))
            if ncols >= P or rows + need > P:
                pending.appendleft(i)
                t += 1
                rows = ncols = 0
                Kt = 0
                colcnt = [0, 0, 0, 0]
                assert t < tiles_max, "out of tiles (rows)"
                continue
            cv = color_l[nlist[i]]
            col = colcnt[cv] * NB + cv
            asg[i] = (t, col, rows, need)
            ku = -(-kmax // need)
            if ku > usedK[t]:
                usedK[t] = ku
            colcnt[cv] += 1
            ncols += 1
            rows += need
        assigns.append(asg)
    usedK = np.array(usedK, np.int64)

    K = usedK
    last_tile = max(int(assigns[c][:, 0].max()) for c in range(NC))
    TILES = last_tile + 1
    for c in range(NC):
        if (assigns[c][:, 0] == TILES - 1).sum() > P - NB:
            TILES += 1
            break
    K = np.maximum(K[:TILES], 1)
    if _os.environ.get("PREPDBG"):
        print("K pre-palette:", list(K))
    # round K up to a palette to lengthen equal-K runs (fewer gather calls)
    _PAL = np.array([1, 2, 3, 4, 5, 6, 8, 10, 12, 16, 20, 24], np.int64)
    K = _PAL[np.searchsorted(_PAL, K)]
    # tile ids are arbitrary: relabel sorted by K desc to merge equal-K runs
    tperm = np.argsort(-K, kind="stable")          # new_t -> old_t
    tinv = np.empty_like(tperm)
    tinv[tperm] = np.arange(TILES)                 # old_t -> new_t
    K = K[tperm]
    for c in range(NC):
        assigns[c][:, 0] = tinv[assigns[c][:, 0]]
    # append an always-empty tile to host the reserved zero rows (pos NPC-4..)
    TILES += 1
    K = np.append(K, 1)
    NPC = TILES * P
    TBL = NC * NPC
    assert TBL // 4 <= 32767, (TILES, TBL)

    # ---- gather groups: runs of equal K ----
    groups = []
    t0 = 0
    while t0 < TILES:
        kt = int(K[t0])
        t1 = t0 + 1
        while (t1 < TILES and int(K[t1]) == kt and (t1 - t0 + 1) <= MAX_GT
               and (t1 - t0 + 1) * kt <= MAX_GSLOT):
            t1 += 1
        groups.append((t0, t1, kt))
        t0 = t1
    slot_base = np.concatenate([[0], np.cumsum(K)])
    KSUM = int(K.sum())
    ktot = NB * KSUM

    pos_of = np.full(N_NODES, -1, np.int64)
    for c in range(NC):
        nodes = per_core_nodes[c]
        asg = assigns[c]
        pos_of[nodes] = asg[:, 0] * P + asg[:, 1]
    pid = core_of * NPC + pos_of

    x_g = np.zeros((TBL, D), np.float32)
    x_g[pid] = x * dinv[:, None]

    g0s = np.array([g[0] for g in groups])
    g1s = np.array([g[1] for g in groups])
    gb_of_tile = np.zeros(TILES, np.int64)
    for gi, (g0, g1, kt) in enumerate(groups):
        gb_of_tile[g0:g1] = gi

    idx_all, F_all, dinvt_all, spool_all = [], [], [], []
    cnts = np.maximum(np.bincount(batch, minlength=N_GRAPHS), 1).astype(np.float32)

    for c in range(NC):
        lo, hi = node_start[c], node_start[c + 1]
        nodes = per_core_nodes[c]
        asg = assigns[c]
        nloc = hi - lo
        tile_of = np.zeros(nloc, np.int64)
        col_of = np.zeros(nloc, np.int64)
        row0_of = np.zeros(nloc, np.int64)
        nrows_of = np.zeros(nloc, np.int64)
        tile_of[nodes - lo] = asg[:, 0]
        col_of[nodes - lo] = asg[:, 1]
        row0_of[nodes - lo] = asg[:, 2]
        nrows_of[nodes - lo] = asg[:, 3]

        m = (a_dst >= lo) & (a_dst < hi)
        es, ed = a_src[m], a_dst[m] - lo
        eb = color[a_src[m]]
        okey = np.lexsort((es, eb, ed))
        es_o, ed_o, eb_o = es[okey], ed[okey], eb[okey]
        grp_key = ed_o * NB + eb_o
        first = np.r_[True, grp_key[1:] != grp_key[:-1]]
        starts = np.where(first, np.arange(grp_key.size), 0)
        rank = np.arange(grp_key.size) - np.maximum.accumulate(starts)
        nr = nrows_of[ed_o]
        row = row0_of[ed_o] + rank % nr
        k = rank // nr
        tt = tile_of[ed_o]
        assert (k < K[tt]).all(), "slot overflow"

        egi = gb_of_tile[tt]
        so = (tt - g0s[egi]) * K[tt] + k
        gbase = 4 * slot_base[g0s[egi]]
        bseg = (g1s[egi] - g0s[egi]) * K[tt]
        i_call = so * P + row
        wcol = 8 * (gbase + eb_o * bseg) + i_call // 16
        idx16 = np.full((16, 8 * ktot), -1, np.int16)
        idx16[i_call % 16, wcol] = (
            (core_of[es_o] * NPC + pos_of[es_o]) // 4).astype(np.int16)

        for gi, (g0, g1, kt) in enumerate(groups):
            gbase = 4 * slot_base[g0]
            bseg = (g1 - g0) * kt
            for b in range(NB):
                c0, c1 = 8 * (gbase + b * bseg), 8 * (gbase + (b + 1) * bseg)
                blk = idx16[:, c0:c1]
                zrow = (c * NPC + (NPC - 4) + b) // 4
                blk[blk < 0] = zrow
        idx_all.append(np.tile(idx16, (8, 1)))

        # fold F: [P rows, TILES*P] (partition-major for group-sliced DMA)
        F = np.zeros((P, TILES * P), np.float32)
        jj = np.arange(nloc)
        for r in range(int(nrows_of.max())):
            sel = nrows_of > r
            F[row0_of[sel] + r, tile_of[sel] * P + col_of[sel]] = 1.0

        F_all.append(F)

        dinvt = np.ones((P, TILES), np.float32)
        dinvt[col_of, tile_of] = dinv[lo:hi]
        dinvt_all.append(dinvt)

        # spool: [P cols, TILES*GPC] partition-major
        spool = np.zeros((P, TILES * GPC), np.float32)
        gl = batch[lo:hi] - c * GPC
        spool[col_of, tile_of * GPC + gl] = 1.0 / cnts[batch[lo:hi]]
        spool_all.append(spool)

    return dict(
        x_g=x_g, idx=idx_all, F=F_all, dinvt=dinvt_all, spool=spool_all,
        K=K, groups=groups, TILES=TILES, NPC=NPC, TBL=TBL, ktot=ktot,
        slot_base=slot_base, KSUM=KSUM, dinv=dinv, pid=pid,
    )


def _build(K, groups, TILES, zero_b1, zero_b2):
    import concourse.bacc as bacc
    import concourse.mybir as mybir
    import concourse.tile as tile
    from concourse import library_config

    f32 = mybir.dt.float32
    bf16 = mybir.dt.bfloat16
    i16 = mybir.dt.int16
    Act = mybir.ActivationFunctionType
    Alu = mybir.AluOpType

    NPC = TILES * P
    TBL = NC * NPC
    KSUM = int(K.sum())
    ktot = NB * KSUM
    slot_base = np.concatenate([[0], np.cumsum(K)])
    WROWS = TBL // 4

    nc = bacc.Bacc("TRN2", target_bir_lowering=False, debug=False,
                   num_devices=NC, num_swdge_queues=4)

    x_g = nc.dram_tensor("x_g", [TBL, D], bf16, kind="ExternalInput")
    idx_in = nc.dram_tensor("idx", [P, 8 * ktot], i16, kind="ExternalInput")
    F_in = nc.dram_tensor("fmat", [P, TILES * P], f32, kind="ExternalInput")
    dinvt_in = nc.dram_tensor("dinvt", [P, TILES], f32, kind="ExternalInput")
    spool_in = nc.dram_tensor("spool", [P, TILES * GPC], f32, kind="ExternalInput")
    w1_in = nc.dram_tensor("w1", [D, D], f32, kind="ExternalInput")
    w2_in = nc.dram_tensor("w2", [D, D], f32, kind="ExternalInput")
    wih_in = nc.dram_tensor("wih", [D, 3 * D], f32, kind="ExternalInput")
    bias_rz_in = nc.dram_tensor("bias_rz", [P, 2], f32, kind="ExternalInput")
    bias_n_in = nc.dram_tensor("bias_n", [P, 2], f32, kind="ExternalInput")
    wlin_in = nc.dram_tensor("wlin", [D, 1], f32, kind="ExternalInput")
    blin_in = nc.dram_tensor("blin", [1, 1], f32, kind="ExternalInput")
    b1_in = nc.dram_tensor("b1b", [P, D], f32, kind="ExternalInput")
    b2_in = nc.dram_tensor("b2b", [P, D], f32, kind="ExternalInput")
    out = nc.dram_tensor("out", [1, GPC], f32, kind="ExternalOutput")

    ag_in = nc.dram_tensor("ag_in", [NPC, D], bf16, kind="Internal")
    h1g = nc.dram_tensor("h1g", [TBL, D], bf16, kind="Internal",
                         addr_space="Shared")

    nc.gpsimd.load_library(library_config.mlp)

    max_bseg = max((g1 - g0) * kt for (g0, g1, kt) in groups)
    max_gt = max(g1 - g0 for (g0, g1, kt) in groups)

    with tile.TileContext(nc) as tc:
        with (
            tc.tile_pool(name="io", bufs=1) as io,
            tc.tile_pool(name="gp", bufs=int(_os.environ.get("GBUFS", "2"))) as gp,
            tc.tile_pool(name="zb", bufs=2) as zbp,
            tc.tile_pool(name="fp", bufs=2) as fpool,
            tc.tile_pool(name="sp", bufs=2) as sp,
            tc.tile_pool(name="wk", bufs=4) as wk,
            tc.tile_pool(name="hw", bufs=1) as hw,
            tc.tile_pool(name="hg", bufs=2) as hgp,
            tc.tile_pool(name="ps_t", bufs=2, space="PSUM") as ps_t,
            tc.tile_pool(name="ps_m", bufs=2, space="PSUM") as ps_m,
            tc.tile_pool(name="ps_pool", bufs=1, space="PSUM") as ps_pool,
            tc.tile_pool(name="ps_h", bufs=2, space="PSUM") as ps_h,
        ):
            idx_t = io.tile([P, 8 * ktot], i16)
            dinv_t = io.tile([P, TILES], f32)
            w1_t = io.tile([D, D], f32)
            w2_t = io.tile([D, D], f32)
            nc.sync.dma_start(out=idx_t[:], in_=idx_in[:])
            nc.sync.dma_start(out=dinv_t[:], in_=dinvt_in[:])
            nc.sync.dma_start(out=w1_t[:], in_=w1_in[:])
            nc.sync.dma_start(out=w2_t[:], in_=w2_in[:])
            b1_t = io.tile([P, D], f32)
            b2_t = io.tile([P, D], f32)
            if not zero_b1:
                nc.sync.dma_start(out=b1_t[:], in_=b1_in[:])
            if not zero_b2:
                nc.sync.dma_start(out=b2_t[:], in_=b2_in[:])

            call_ctr = [0]

            def layer(table, w_t, is_l1):
                pool_ps = None
                if not is_l1:
                    pool_ps = ps_pool.tile([P, GPC], f32, space="PSUM")
                for gi, (g0, g1, kt) in enumerate(groups):
                    GT = g1 - g0
                    bseg = GT * kt
                    gbase = 4 * int(slot_base[g0])
                    g = gp.tile([P, 4 * max_bseg * D], bf16, tag="g")
                    zbs = []
                    for b in range(NB):
                        win = table.rearrange("(r four) f -> r (four f)",
                                              four=4)[:WROWS, b * D:(b + 1) * D]
                        s0 = 0
                        while s0 < bseg:
                            kk = min(bseg - s0, MAXIDX // P)
                            wbase = 8 * (gbase + b * bseg + s0)
                            nc.gpsimd.dma_gather(
                                out_ap=g[:, (b * bseg + s0) * D:
                                         (b * bseg + s0 + kk) * D].rearrange(
                                    "p (k f) -> p k f", k=kk),
                                in_ap=win,
                                idxs_ap=idx_t[:, wbase:wbase + 8 * kk],
                                num_idxs=kk * P,
                                num_idxs_reg=kk * P,
                                elem_size=D,
                                elem_step=4 * D,
                                single_packet=False,
                                queue_num=call_ctr[0] % 4,
                            )
                            call_ctr[0] += 1
                            s0 += kk
                        # partial reduce for this bank over the whole group
                        zb = zbp.tile([P, max_gt * D], f32, tag=f"zb{b}")
                        gseg = g[:, b * bseg * D:(b + 1) * bseg * D]
                        nc.vector.tensor_reduce(
                            out=zb[:, :GT * D],
                            in_=gseg.rearrange("p (t k f) -> p t f k", t=GT, k=kt),
                            axis=mybir.AxisListType.X, op=Alu.add)
                        zbs.append(zb)

                    Fg = fpool.tile([P, max_gt * P], f32, tag="F")
                    nc.sync.dma_start(out=Fg[:, :GT * P],
                                      in_=F_in[:, g0 * P:g1 * P])
                    spt = None
                    if not is_l1:
                        spt = sp.tile([P, max_gt * GPC], f32, tag="sp")
                        nc.sync.dma_start(out=spt[:, :GT * GPC],
                                          in_=spool_in[:, g0 * GPC:g1 * GPC])
                    hgrp = hgp.tile([P, max_gt * D], bf16 if is_l1 else f32,
                                    tag="hg1" if is_l1 else "hg2")
                    for tl in range(GT):
                        t = g0 + tl
                        zf_ps = ps_t.tile([P, P], f32, space="PSUM", tag="zf")
                        for b in range(NB):
                            nc.tensor.matmul(
                                out=zf_ps[:],
                                lhsT=zbs[b][:, tl * D:(tl + 1) * D],
                                rhs=Fg[:, tl * P:(tl + 1) * P],
                                start=(b == 0), stop=(b == NB - 1))
                        zf = wk.tile([P, P], f32, tag="zf")
                        nc.scalar.copy(out=zf[:], in_=zf_ps[:])
                        h_ps = ps_m.tile([P, D], f32, space="PSUM", tag="h")
                        nc.tensor.matmul(out=h_ps[:], lhsT=zf[:], rhs=w_t[:],
                                         start=True, stop=True)
                        sc = dinv_t[:, t:t + 1]
                        if is_l1:
                            if zero_b1:
                                ht = wk.tile([P, D], f32, tag="ht")
                                nc.scalar.activation(out=ht[:], in_=h_ps[:],
                                                     func=Act.Relu, scale=sc)
                            else:
                                hf = wk.tile([P, D], f32, tag="hf")
                                nc.vector.tensor_scalar_mul(hf[:], h_ps[:], sc)
                                nc.vector.tensor_tensor(out=hf[:], in0=hf[:],
                                                        in1=b1_t[:], op=Alu.add)
                                ht = wk.tile([P, D], f32, tag="ht")
                                nc.scalar.activation(out=ht[:], in_=hf[:],
                                                     func=Act.Relu)
                            h2 = hgrp[:, tl * D:(tl + 1) * D]
                            nc.scalar.mul(out=h2, in_=ht[:], mul=sc)
                        else:
                            h2 = hgrp[:, tl * D:(tl + 1) * D]
                            if zero_b2:
                                nc.scalar.mul(out=h2, in_=h_ps[:], mul=sc)
                            else:
                                hf = wk.tile([P, D], f32, tag="hf")
                                nc.vector.tensor_scalar_mul(hf[:], h_ps[:], sc)
                                nc.vector.tensor_tensor(out=h2, in0=hf[:],
                                                        in1=b2_t[:], op=Alu.add)
                            nc.tensor.matmul(out=pool_ps[:], lhsT=h2,
                                             rhs=spt[:, tl * GPC:(tl + 1) * GPC],
                                             start=(t == 0), stop=(t == TILES - 1))
                    if is_l1:
                        nc.sync.dma_start(
                            out=ag_in[g0 * P:g1 * P, :].rearrange(
                                "(t p) f -> p t f", p=P),
                            in_=hgrp[:, :GT * D].rearrange(
                                "p (t f) -> p t f", t=GT))
                return pool_ps

            layer(x_g[:], w1_t, True)
            nc.gpsimd.collective_compute(
                "AllGather", mybir.AluOpType.bypass,
                replica_groups=[list(range(NC))],
                ins=[ag_in[:]], outs=[h1g[:]],
            )
            pool_ps = layer(h1g[:], w2_t, False)

            # ---- head (f32, as v1) ----
            wih_t = io.tile([D, 3 * D], f32)
            brz_t = io.tile([P, 2], f32)
            bn_t = io.tile([P, 2], f32)
            wlin_t = io.tile([D, 1], f32)
            blin_t = io.tile([1, 1], f32)
            ones_m = io.tile([P, 1], f32)
            eps_t = io.tile([1, 1], f32)
            ones_r = io.tile([1, P], f32)
            nc.sync.dma_start(out=wih_t[:], in_=wih_in[:])
            nc.sync.dma_start(out=brz_t[:], in_=bias_rz_in[:])
            nc.sync.dma_start(out=bn_t[:], in_=bias_n_in[:])
            nc.sync.dma_start(out=wlin_t[:], in_=wlin_in[:])
            nc.sync.dma_start(out=blin_t[:], in_=blin_in[:])
            nc.vector.memset(ones_m[:], 1.0 / P)
            nc.vector.memset(eps_t[:], EPS)
            nc.vector.memset(ones_r[:], 1.0)

            hd = hw.tile([P, GPC], f32, tag="hd")
            nc.scalar.copy(out=hd[:], in_=pool_ps[:])

            def gate_mm(sl):
                ps = ps_h.tile([P, GPC], f32, space="PSUM", tag="hps")
                nc.tensor.matmul(out=ps[:], lhsT=wih_t[:, sl * D:(sl + 1) * D],
                                 rhs=hd[:], start=True, stop=True)
                return ps

            r = hw.tile([P, GPC], f32, tag="r")
            nc.scalar.activation(out=r[:], in_=gate_mm(0)[:], func=Act.Sigmoid,
                                 bias=brz_t[:, 0:1], scale=1.0)
            zz = hw.tile([P, GPC], f32, tag="zz")
            nc.scalar.activation(out=zz[:], in_=gate_mm(1)[:], func=Act.Sigmoid,
                                 bias=brz_t[:, 1:2], scale=1.0)
            nps = gate_mm(2)
            rb = hw.tile([P, GPC], f32, tag="rb")
            nc.vector.tensor_scalar_mul(rb[:], r[:], bn_t[:, 1:2])
            t1 = hw.tile([P, GPC], f32, tag="t1")
            nc.vector.tensor_tensor(out=t1[:], in0=nps[:], in1=rb[:], op=Alu.add)
            n_t = hw.tile([P, GPC], f32, tag="nt")
            nc.scalar.activation(out=n_t[:], in_=t1[:], func=Act.Tanh,
                                 bias=bn_t[:, 0:1], scale=1.0)
            zn = hw.tile([P, GPC], f32, tag="zn")
            nc.vector.tensor_tensor(out=zn[:], in0=zz[:], in1=n_t[:], op=Alu.mult)
            hr = hw.tile([P, GPC], f32, tag="hr")
            nc.vector.tensor_tensor(out=hr[:], in0=n_t[:], in1=zn[:],
                                    op=Alu.subtract)
            nc.scalar.activation(out=hr[:], in_=hr[:], func=Act.Relu)

            mu_ps = ps_h.tile([1, GPC], f32, space="PSUM", tag="hps")
            nc.tensor.matmul(out=mu_ps[:], lhsT=ones_m[:], rhs=hr[:],
                             start=True, stop=True)
            mu = hw.tile([1, GPC], f32, tag="mu")
            nc.scalar.copy(out=mu[:], in_=mu_ps[:])
            mub_ps = ps_h.tile([P, GPC], f32, space="PSUM", tag="hps")
            nc.tensor.matmul(out=mub_ps[:], lhsT=ones_r[:], rhs=mu[:],
                             start=True, stop=True)
            dmu = hw.tile([P, GPC], f32, tag="dmu")
            nc.vector.tensor_tensor(out=dmu[:], in0=hr[:], in1=mub_ps[:],
                                    op=Alu.subtract)
            d2 = hw.tile([P, GPC], f32, tag="d2")
            nc.scalar.activation(out=d2[:], in_=dmu[:], func=Act.Square)
            var_ps = ps_h.tile([1, GPC], f32, space="PSUM", tag="hps")
            nc.tensor.matmul(out=var_ps[:], lhsT=ones_m[:], rhs=d2[:],
                             start=True, stop=True)
            std = hw.tile([1, GPC], f32, tag="std")
            nc.scalar.activation(out=std[:], in_=var_ps[:], func=Act.Sqrt,
                                 bias=eps_t[:, 0:1])
            rstd = hw.tile([1, GPC], f32, tag="rstd")
            nc.vector.reciprocal(rstd[:], std[:])
            rsb_ps = ps_h.tile([P, GPC], f32, space="PSUM", tag="hps")
            nc.tensor.matmul(out=rsb_ps[:], lhsT=ones_r[:], rhs=rstd[:],
                             start=True, stop=True)
            gn = hw.tile([P, GPC], f32, tag="gn")
            nc.vector.tensor_tensor(out=gn[:], in0=dmu[:], in1=rsb_ps[:],
                                    op=Alu.mult)
            o_ps = ps_h.tile([1, GPC], f32, space="PSUM", tag="hps")
            nc.tensor.matmul(out=o_ps[:], lhsT=wlin_t[:], rhs=gn[:],
                             start=True, stop=True)
            o_sb = hw.tile([1, GPC], f32, tag="o")
            nc.scalar.activation(out=o_sb[:], in_=o_ps[:], func=Act.Identity,
                                 bias=blin_t[:, 0:1], scale=1.0)
            nc.sync.dma_start(out=out[:], in_=o_sb[:])

    nc.compile()
    return nc


def _bfnp():
    import ml_dtypes
    return ml_dtypes.bfloat16


def kernel(**inputs):
    x = np.ascontiguousarray(np.asarray(inputs["x"], dtype=np.float32))
    ei = np.asarray(inputs["edge_index"]).astype(np.int64)
    batch = np.asarray(inputs["batch"]).astype(np.int64)
    W1 = np.asarray(inputs["W1"], np.float32)
    b1 = np.asarray(inputs["b1"], np.float32)
    W2 = np.asarray(inputs["W2"], np.float32)
    b2 = np.asarray(inputs["b2"], np.float32)
    W_ih = np.asarray(inputs["W_ih"], np.float32)
    b_ih = np.asarray(inputs["b_ih"], np.float32)
    b_hh = np.asarray(inputs["b_hh"], np.float32)
    W_lin = np.asarray(inputs["W_lin"], np.float32)
    b_lin = np.asarray(inputs["b_lin"], np.float32)

    prep = _prep(x, ei[0], ei[1], batch)

    zero_b1 = not np.any(b1)
    zero_b2 = not np.any(b2)
    key = (prep["TILES"], prep["ktot"], bytes(prep["K"]), zero_b1, zero_b2)
    if key not in _CACHE:
        _CACHE[key] = _build(prep["K"], prep["groups"], prep["TILES"],
                             zero_b1, zero_b2)
    nc = _CACHE[key]

    wih = np.concatenate([W_ih[i * D:(i + 1) * D, :].T for i in range(3)],
                         axis=1).astype(np.float32)
    bias_rz = np.stack([b_ih[0:D] + b_hh[0:D], b_ih[D:2 * D] + b_hh[D:2 * D]],
                       axis=1).astype(np.float32)
    bias_n = np.stack([b_ih[2 * D:], b_hh[2 * D:]], axis=1).astype(np.float32)
    b1b = np.tile(b1[None, :], (P, 1)).astype(np.float32)
    b2b = np.tile(b2[None, :], (P, 1)).astype(np.float32)

    in_maps = []
    for c in range(NC):
        in_maps.append({
            "x_g": prep["x_g"].astype(_bfnp()),
            "idx": prep["idx"][c],
            "fmat": prep["F"][c],
            "dinvt": prep["dinvt"][c],
            "spool": prep["spool"][c],
            "w1": W1, "w2": W2,
            "wih": wih, "bias_rz": bias_rz, "bias_n": bias_n,
            "wlin": W_lin.T.astype(np.float32).reshape(D, 1),
            "blin": b_lin.reshape(1, 1).astype(np.float32),
            "b1b": b1b, "b2b": b2b,
        })

    global _last_in_maps
    _last_in_maps = in_maps
    from concourse.bass_utils import run_bass_kernel_spmd
    res = run_bass_kernel_spmd(nc, in_maps, core_ids=list(range(NC)))
    out = np.concatenate([res.results[c]["out"][0] for c in range(NC)])
    return out.reshape(N_GRAPHS, 1).astype(np.float32)
